# revision 7
# baseline (speedup 1.0000x reference)
"""LRU layer Trainium2 kernel — data-parallel over batch (1 row/core, 8 cores).

Per-core pipeline (16 chunks of 128 timesteps), software-pipelined with a
2-chunk skew so the in-order PE stream is dense:
  iteration it:  inproj(it) | cumsum(it-1) | outproj(it-2)

The complex diagonal scan s_t = lam*s_{t-1} + u_t is computed as
  s[j] = lam^j * ( cumsum_j( lam^-q * u[q] ) + carry )
which turns the recurrence into matmuls (cumsum == upper-tri ones matrix)
plus elementwise complex modulation.

The input-bias trajectory (b_in -> scan -> outproj + b_out) is input-
independent, so it is precomputed on the host as yb[t, d] and folded into
the residual: kernel receives xb = x + yb and no bias matmuls are issued.
"""
import json
import numpy as np
import ml_dtypes

B, L, D, H = 8, 2048, 512, 1024
C = 128           # time chunk
NBLK = 8          # H blocks of 128
NCHUNK = L // C
EPS = 1e-5
BF = ml_dtypes.bfloat16

_COMPILED = {}


def _split_multi_waits(bir_bytes):
    """This walrus build accepts only one sync-wait per instruction; hoist
    extras onto single-wait EventSemaphore carriers on the same engine."""
    d = json.loads(bir_bytes)
    ctr = 0
    for fn in d.get("functions", []):
        for bb in fn.get("blocks", []):
            out = []
            for ins in bb.get("instructions", []):
                si = ins.get("sync_info")
                ow = (si.get("on_wait") if si else None) or []
                if len(ow) > 1:
                    for w in ow[:-1]:
                        ctr += 1
                        out.append({
                            "debug": ins.get("debug", 0),
                            "engine": ins["engine"],
                            "ins": [], "outs": [],
                            "name": f"WSPLIT-{ctr}",
                            "opcode": "EventSemaphore",
                            "sync_info": {"on_update": [], "on_wait": [w]},
                        })
                    si["on_wait"] = [ow[-1]]
                out.append(ins)
            bb["instructions"] = out
    return json.dumps(d).encode()


def _prep_consts(params_log, W_in_re, W_in_im, W_out_re, W_out_im):
    nu, theta, gamma = np.exp(np.asarray(params_log, np.float64))
    lam = np.exp(-nu + 1j * theta)                        # (H,) complex128
    j = np.arange(C)[:, None]
    lamNeg = lam[None, :] ** (-j)                         # (128, H) time-layout
    lpr = np.empty((128, NBLK * 128)); lpi = np.empty_like(lpr)
    for b in range(NBLK):
        blk = lam[b * 128:(b + 1) * 128][:, None] ** np.arange(C)[None, :]
        lpr[:, b * 128:(b + 1) * 128] = blk.real
        lpi[:, b * 128:(b + 1) * 128] = blk.imag
    lam128 = (lam ** 128).reshape(NBLK, 128).T            # (128, 8) blocked
    Wr = np.asarray(W_in_re, np.float64) * gamma[:, None]
    Wi = np.asarray(W_in_im, np.float64) * gamma[:, None]
    return dict(
        WtT_re=Wr.T.astype(np.float32).astype(BF),        # (512, 1024)
        WtT_im=Wi.T.astype(np.float32).astype(BF),
        lnr=lamNeg.real.astype(np.float32).astype(BF),
        lni=lamNeg.imag.astype(np.float32).astype(BF),
        lpr=lpr.astype(np.float32).astype(BF),
        lpi=lpi.astype(np.float32).astype(BF),
        l128_re=lam128.real.astype(np.float32),
        l128_im=lam128.imag.astype(np.float32),
        TT=np.triu(np.ones((C, C), np.float32)).astype(BF),
        WoT_re=np.ascontiguousarray(np.asarray(W_out_re, np.float32).T).astype(BF),
        WoT_imn=np.ascontiguousarray(-np.asarray(W_out_im, np.float32).T).astype(BF),
    )


def _bias_residual(params_log, b_in_re, b_in_im, W_out_re, W_out_im, b_out_re):
    """yb[t, d] = Re( (b~ * sum_{p<=t} lam^p) @ W_out^T ) + b_out  — the
    input-independent trajectory of b_in through scan + outproj."""
    nu, theta, gamma = np.exp(np.asarray(params_log, np.float64))
    loglam = -nu + 1j * theta                              # (H,)
    bt = (np.asarray(b_in_re, np.float64)
          + 1j * np.asarray(b_in_im, np.float64)) * gamma  # (H,)
    t = np.arange(1, L + 1)[:, None]                       # (L, 1): power t+1
    lam_tp1 = np.exp(t * loglam[None, :])                  # (L, H)
    geo = (1.0 - lam_tp1) / (1.0 - np.exp(loglam))[None, :]
    sb = bt[None, :] * geo                                 # (L, H) complex
    yb = (sb.real @ np.asarray(W_out_re, np.float64).T
          - sb.imag @ np.asarray(W_out_im, np.float64).T
          + np.asarray(b_out_re, np.float64)[None, :])
    return yb.astype(np.float32)                           # (L, D)


def _build(consts):
    import concourse.bass as bass
    import concourse.tile as tile
    from concourse import mybir

    f32 = mybir.dt.float32
    bf16 = mybir.dt.bfloat16
    AF = mybir.ActivationFunctionType
    OP = mybir.AluOpType

    nc = bass.Bass()
    x_in = nc.declare_dram_parameter("x", [L, D], f32, isOutput=False)
    xT_in = nc.declare_dram_parameter("xT", [D, L], bf16, isOutput=False)
    y_out = nc.declare_dram_parameter("y", [L, D], f32, isOutput=True)

    dr = {k: nc.inline_tensor(v, name=f"c_{k}") for k, v in consts.items()}

    with tile.TileContext(nc) as tc:
        with tc.tile_pool(name="cst", bufs=1) as cst, \
             tc.tile_pool(name="ioT", bufs=3) as ioT, \
             tc.tile_pool(name="iox", bufs=6) as iox, \
             tc.tile_pool(name="ioo", bufs=3) as ioo, \
             tc.tile_pool(name="wk", bufs=3) as wk, \
             tc.tile_pool(name="sc", bufs=3) as sc, \
             tc.tile_pool(name="sm", bufs=4) as sm, \
             tc.tile_pool(name="psu", bufs=4, space="PSUM") as psu, \
             tc.tile_pool(name="psw", bufs=3, space="PSUM") as psw, \
             tc.tile_pool(name="psy", bufs=1, space="PSUM") as psy:

            # ---- load constants to SBUF ----
            # Issue order matters: only what inproj/premod of chunk 0 needs
            # goes first, so PE can start while the rest streams in.
            WtT = {}
            for part in ("re", "im"):
                for db in range(4):
                    t = cst.tile([128, H], bf16, name=f"WtT_{part}_{db}")
                    nc.sync.dma_start(out=t, in_=dr[f"WtT_{part}"][db * 128:(db + 1) * 128, :])
                    WtT[part, db] = t
            cst_t = {}
            for k in ("lnr", "lni", "TT"):
                shp = [128, H] if k != "TT" else [128, C]
                cst_t[k] = cst.tile(shp, bf16, name=f"t_{k}")
                nc.sync.dma_start(out=cst_t[k], in_=dr[k][:, :])
            eps_t = cst.tile([128, 1], f32, name="eps_t")
            nc.vector.memset(eps_t, EPS)

            def load_late_consts():
                WoT = {}
                for part, key in (("re", "WoT_re"), ("im", "WoT_imn")):
                    t = cst.tile([128, NBLK, D], bf16, name=f"WoT_{part}")
                    nc.sync.dma_start(out=t, in_=dr[key].rearrange("(b p) d -> p b d", p=128))
                    WoT[part] = t
                for k in ("lpr", "lpi"):
                    cst_t[k] = cst.tile([128, H], bf16, name=f"t_{k}")
                    nc.sync.dma_start(out=cst_t[k], in_=dr[k][:, :])
                for k in ("l128_re", "l128_im"):
                    cst_t[k] = cst.tile([128, NBLK], f32, name=f"t_{k}")
                    nc.sync.dma_start(out=cst_t[k], in_=dr[k][:, :])
                return WoT

            # persistent carry tiles, double-buffered by chunk parity
            c_t = {}
            for par in (0, 1):
                for part in ("re", "im"):
                    c_t[par, part] = cst.tile([128, NBLK], f32, name=f"carry_{par}_{part}")
            nc.vector.memset(c_t[0, "re"], 0.0)
            nc.vector.memset(c_t[0, "im"], 0.0)

            # per-chunk live tiles, keyed by chunk index
            xT_t, x_t, u_ps, u_sb, v_sb, w_ps, wp_sb, s_sb, y_ps = \
                {}, {}, {}, {}, {}, {}, {}, {}, {}

            def issue_load(k):
                r0 = k * C
                t = ioT.tile([128, 4, 128], bf16, tag="xT")
                nc.sync.dma_start(
                    out=t,
                    in_=xT_in[:, r0:r0 + C].rearrange("(db p) t -> p db t", p=128))
                xT_t[k] = t
                t = iox.tile([128, D], f32, tag="x")
                nc.sync.dma_start(out=t, in_=x_in[r0:r0 + C, :])
                x_t[k] = t

            def issue_inproj(k):
                # db-outer so each LDWEIGHTS (stationary xT block) feeds 2 MMs
                for part in ("re", "im"):
                    pts = []
                    for nh in range(2):
                        pts.append(psu.tile([128, 512], f32, tag="psu", name="u_ps"))
                    for db in range(4):
                        for nh in range(2):
                            nc.tensor.matmul(pts[nh], xT_t[k][:, db, :],
                                             WtT[part, db][:, nh * 512:(nh + 1) * 512],
                                             start=(db == 0), stop=(db == 3))
                    u_ps[k, part] = pts

            def issue_uevict(k):
                for part in ("re", "im"):
                    t = wk.tile([128, H], bf16, tag=f"u_{part}")
                    for nh in range(2):
                        nc.scalar.activation(out=t[:, nh * 512:(nh + 1) * 512],
                                             in_=u_ps[k, part][nh], func=AF.Identity)
                    u_sb[k, part] = t

            def issue_premod(k):
                u = u_sb
                t1 = sc.tile([128, H], bf16, tag="t1")
                t2 = sc.tile([128, H], bf16, tag="t2")
                nc.vector.tensor_mul(t1, cst_t["lnr"], u[k, "re"])
                nc.vector.tensor_mul(t2, cst_t["lni"], u[k, "im"])
                v_re = wk.tile([128, H], bf16, tag="v_re")
                nc.vector.tensor_sub(v_re, t1, t2)
                t3 = sc.tile([128, H], bf16, tag="t3")
                t4 = sc.tile([128, H], bf16, tag="t4")
                nc.vector.tensor_mul(t3, cst_t["lnr"], u[k, "im"])
                nc.vector.tensor_mul(t4, cst_t["lni"], u[k, "re"])
                v_im = wk.tile([128, H], bf16, tag="v_im")
                nc.gpsimd.tensor_add(v_im, t3, t4)
                v_sb[k] = {"re": v_re, "im": v_im}

            def issue_cumsum(k):
                # transposing cumsum: w^T[h, j] = sum_{q<=j} v[q, h]
                for part in ("re", "im"):
                    for half in range(2):
                        pt = psw.tile([128, 512], f32, tag="psw")
                        for bq in range(4):
                            b = half * 4 + bq
                            nc.tensor.matmul(pt[:, bq * 128:(bq + 1) * 128],
                                             v_sb[k][part][:, b * 128:(b + 1) * 128],
                                             cst_t["TT"], start=True, stop=True)
                        w_ps[k, part, half] = pt

            def issue_carry(k):
                # c_next = lam^128 * (w'[:,127] + c_cur); off the evict path
                cur, nxt = k % 2, (k + 1) % 2
                if k >= NCHUNK - 1:
                    return
                wcol = {}
                for part in ("re", "im"):
                    t = sm.tile([128, NBLK], f32, tag=f"wcol_{part}")
                    for half in range(2):
                        nc.vector.tensor_add(
                            t[:, half * 4:(half + 1) * 4],
                            w_ps[k, part, half][:, 127::128],
                            c_t[cur, part][:, half * 4:(half + 1) * 4])
                    wcol[part] = t
                m1 = sm.tile([128, NBLK], f32, tag="m1")
                m2 = sm.tile([128, NBLK], f32, tag="m2")
                nc.vector.tensor_mul(m1, cst_t["l128_re"], wcol["re"])
                nc.vector.tensor_mul(m2, cst_t["l128_im"], wcol["im"])
                nc.vector.tensor_sub(c_t[nxt, "re"], m1, m2)
                m3 = sm.tile([128, NBLK], f32, tag="m3")
                m4 = sm.tile([128, NBLK], f32, tag="m4")
                nc.vector.tensor_mul(m3, cst_t["l128_re"], wcol["im"])
                nc.vector.tensor_mul(m4, cst_t["l128_im"], wcol["re"])
                nc.vector.tensor_add(c_t[nxt, "im"], m3, m4)

            def issue_wevict(k):
                # carry-add + evict (ACT Identity w/ per-partition bias)
                cur = k % 2
                for part in ("re", "im"):
                    t = wk.tile([128, H], bf16, tag=f"wp_{part}")
                    for b in range(NBLK):
                        nc.scalar.activation(
                            out=t[:, b * 128:(b + 1) * 128],
                            in_=w_ps[k, part, b // 4][:, (b % 4) * 128:(b % 4 + 1) * 128],
                            func=AF.Identity,
                            bias=c_t[cur, part][:, b:b + 1])
                    wp_sb[k, part] = t

            def issue_postmod(k):
                p1 = sc.tile([128, H], bf16, tag="p1")
                p2 = sc.tile([128, H], bf16, tag="p2")
                nc.vector.tensor_mul(p1, cst_t["lpr"], wp_sb[k, "re"])
                nc.vector.tensor_mul(p2, cst_t["lpi"], wp_sb[k, "im"])
                s_re = wk.tile([128, H], bf16, tag="s_re")
                nc.vector.tensor_sub(s_re, p1, p2)
                p3 = sc.tile([128, H], bf16, tag="p3")
                p4 = sc.tile([128, H], bf16, tag="p4")
                nc.vector.tensor_mul(p3, cst_t["lpr"], wp_sb[k, "im"])
                nc.vector.tensor_mul(p4, cst_t["lpi"], wp_sb[k, "re"])
                s_im = wk.tile([128, H], bf16, tag="s_im")
                nc.gpsimd.tensor_add(s_im, p3, p4)
                s_sb[k] = {"re": s_re, "im": s_im}

            def issue_outproj(k):
                pt = psy.tile([128, 512], f32, tag="psy")
                first = True
                for part in ("re", "im"):
                    for b in range(NBLK):
                        nc.tensor.matmul(pt, s_sb[k][part][:, b * 128:(b + 1) * 128],
                                         WoT[part][:, b, :], start=first,
                                         stop=(part == "im" and b == NBLK - 1))
                        first = False
                y_ps[k] = pt

            def issue_ln(k):
                r0 = k * C
                ysum = sm.tile([128, 1], f32, tag="ysum")
                y_sb = iox.tile([128, D], f32, tag="ysb")
                nc.vector.scalar_tensor_tensor(out=y_sb, in0=y_ps[k], scalar=0.0,
                                               in1=x_t[k], op0=OP.bypass, op1=OP.add,
                                               accum_out=ysum)
                negmu = sm.tile([128, 1], f32, tag="negmu")
                nc.scalar.mul(negmu, ysum, -1.0 / D)
                sq = sc.tile([128, D], bf16, tag="sq")
                ss = sm.tile([128, 1], f32, tag="ss")
                nc.scalar.activation(out=sq, in_=y_sb, func=AF.Square,
                                     bias=negmu, accum_out=ss)
                sd = sm.tile([128, 1], f32, tag="sd")
                nc.scalar.activation(out=sd, in_=ss, func=AF.Sqrt,
                                     bias=eps_t, scale=1.0 / D)
                rstd = sm.tile([128, 1], f32, tag="rstd")
                nc.vector.reciprocal(rstd, sd)
                o_sb = ioo.tile([128, D], f32, tag="osb")
                nc.vector.tensor_scalar(out=o_sb, in0=y_sb, scalar1=negmu,
                                        scalar2=rstd, op0=OP.add, op1=OP.mult)
                nc.sync.dma_start(out=y_out[r0:r0 + C, :], in_=o_sb)

            issue_load(0)
            issue_load(1)
            WoT = load_late_consts()
            for it in range(NCHUNK + 2):
                if it < NCHUNK:
                    if it + 2 < NCHUNK:
                        issue_load(it + 2)
                    issue_inproj(it)
                    issue_uevict(it)
                    issue_premod(it)
                k1 = it - 1
                if 0 <= k1 < NCHUNK:
                    issue_cumsum(k1)
                    issue_carry(k1)
                    issue_wevict(k1)
                    issue_postmod(k1)
                k2 = it - 2
                if 0 <= k2 < NCHUNK:
                    issue_outproj(k2)
                    issue_ln(k2)

    orig = type(nc).to_json_bytes
    nc.to_json_bytes = lambda: _split_multi_waits(orig(nc))
    return nc


def _reference_fallback(x, mask, params_log, W_in_re, W_in_im, b_in_re, b_in_im,
                        W_out_re, W_out_im, b_out_re, b_out_im, ln_w, ln_b):
    nu, theta, gamma = np.exp(np.asarray(params_log, np.float64))
    lam = np.exp(-nu + 1j * theta).astype(np.complex64)[None, :]
    W_in = (np.asarray(W_in_re) + 1j * np.asarray(W_in_im)).astype(np.complex64)
    b_in = (np.asarray(b_in_re) + 1j * np.asarray(b_in_im)).astype(np.complex64)
    W_out = (np.asarray(W_out_re) + 1j * np.asarray(W_out_im)).astype(np.complex64)
    h = (np.asarray(x).astype(np.complex64) @ W_in.T + b_in) * gamma.astype(np.float32)
    Bn, Ln, Dn = h.shape
    log2_L = int(np.ceil(np.log2(Ln)))
    m_ = np.asarray(mask, np.float32)
    lamb = lam.copy()
    for i in range(1, log2_L + 1):
        l = 2 ** i
        hh = h.reshape(Bn * Ln // l, l, Dn)
        mm = m_.reshape(Bn * Ln // l, l)
        h1, h2 = hh[:, :l // 2], hh[:, l // 2:]
        if i > 1:
            lamb = np.concatenate([lamb, lamb * lamb[-1]], axis=0)
        h2 = h2 + lamb * h1[:, -1:] * mm[:, l // 2 - 1:l // 2, None]
        h = np.concatenate([h1, h2], axis=1)
    h = h.reshape(Bn, Ln, Dn)
    y = (h @ W_out.T + (np.asarray(b_out_re) + 1j * np.asarray(b_out_im))).real + np.asarray(x)
    mean = y.mean(-1, keepdims=True)
    var = y.var(-1, keepdims=True)
    return ((y - mean) / np.sqrt(var + EPS) * np.asarray(ln_w) + np.asarray(ln_b)).astype(np.float32)


def kernel(x, mask, params_log, W_in_re, W_in_im, b_in_re, b_in_im,
           W_out_re, W_out_im, b_out_re, b_out_im, ln_w, ln_b, **_):
    x = np.asarray(x, np.float32)
    if not (np.all(np.asarray(mask) == 1.0) and np.allclose(ln_w, 1.0)
            and np.allclose(ln_b, 0.0)):
        return _reference_fallback(x, mask, params_log, W_in_re, W_in_im, b_in_re,
                                   b_in_im, W_out_re, W_out_im, b_out_re, b_out_im,
                                   ln_w, ln_b)
    from concourse.bass_utils import run_bass_kernel_spmd

    consts = _prep_consts(params_log, W_in_re, W_in_im, W_out_re, W_out_im)
    yb = _bias_residual(params_log, b_in_re, b_in_im, W_out_re, W_out_im, b_out_re)

    key = hash((np.asarray(params_log, np.float64).tobytes(),
                np.asarray(W_in_re, np.float32).tobytes(),
                np.asarray(W_out_re, np.float32).tobytes()))
    nc = _COMPILED.get(key)
    if nc is None:
        nc = _build(consts)
        _COMPILED[key] = nc

    in_maps = []
    for b in range(B):
        xT = np.ascontiguousarray(x[b].T).astype(BF)
        in_maps.append({"x": np.ascontiguousarray(x[b] + yb), "xT": xT})
    res = run_bass_kernel_spmd(nc, in_maps, core_ids=list(range(B)))
    return np.stack([res.results[b]["y"] for b in range(B)]).astype(np.float32)


# revision 12
# speedup vs baseline: 1.1914x; 1.1914x over previous
"""LRU layer Trainium2 kernel — data-parallel over batch (1 row/core, 8 cores).

Per-core pipeline (16 chunks of 128 timesteps), software-pipelined with a
2-chunk skew so the in-order PE stream is dense:
  iteration it:  inproj(it) | cumsum(it-1) | outproj(it-2)

The complex diagonal scan s_t = lam*s_{t-1} + u_t is computed as
  s[j] = lam^j * ( cumsum_j( lam^-q * u[q] ) + carry )
which turns the recurrence into matmuls (cumsum == upper-tri ones matrix)
plus elementwise complex modulation.

The input-bias trajectory (b_in -> scan -> outproj + b_out) is input-
independent, so it is precomputed on the host as yb[t, d] and folded into
the residual: kernel receives xb = x + yb and no bias matmuls are issued.
"""
import json
import numpy as np
import ml_dtypes

B, L, D, H = 8, 2048, 512, 1024
C = 128           # time chunk
NBLK = 8          # H blocks of 128
NCHUNK = L // C
EPS = 1e-5
BF = ml_dtypes.bfloat16

_COMPILED = {}


def _split_multi_waits(bir_bytes):
    """This walrus build accepts only one sync-wait per instruction; hoist
    extras onto single-wait EventSemaphore carriers on the same engine."""
    d = json.loads(bir_bytes)
    ctr = 0
    for fn in d.get("functions", []):
        for bb in fn.get("blocks", []):
            out = []
            for ins in bb.get("instructions", []):
                si = ins.get("sync_info")
                ow = (si.get("on_wait") if si else None) or []
                if len(ow) > 1:
                    for w in ow[:-1]:
                        ctr += 1
                        out.append({
                            "debug": ins.get("debug", 0),
                            "engine": ins["engine"],
                            "ins": [], "outs": [],
                            "name": f"WSPLIT-{ctr}",
                            "opcode": "EventSemaphore",
                            "sync_info": {"on_update": [], "on_wait": [w]},
                        })
                    si["on_wait"] = [ow[-1]]
                out.append(ins)
            bb["instructions"] = out
    return json.dumps(d).encode()


def _prep_consts(params_log, W_in_re, W_in_im, W_out_re, W_out_im):
    nu, theta, gamma = np.exp(np.asarray(params_log, np.float64))
    lam = np.exp(-nu + 1j * theta)                        # (H,) complex128
    j = np.arange(C)[:, None]
    lamNeg = lam[None, :] ** (-j)                         # (128, H) time-layout
    lpr = np.empty((128, NBLK * 128)); lpi = np.empty_like(lpr)
    for b in range(NBLK):
        blk = lam[b * 128:(b + 1) * 128][:, None] ** np.arange(C)[None, :]
        lpr[:, b * 128:(b + 1) * 128] = blk.real
        lpi[:, b * 128:(b + 1) * 128] = blk.imag
    lam128 = (lam ** 128).reshape(NBLK, 128).T            # (128, 8) blocked
    Wr = np.asarray(W_in_re, np.float64) * gamma[:, None]
    Wi = np.asarray(W_in_im, np.float64) * gamma[:, None]
    return dict(
        WtT_re=Wr.T.astype(np.float32).astype(BF),        # (512, 1024)
        WtT_im=Wi.T.astype(np.float32).astype(BF),
        lnr=lamNeg.real.astype(np.float32).astype(BF),
        lni=lamNeg.imag.astype(np.float32).astype(BF),
        lpr=lpr.astype(np.float32).astype(BF),
        lpi=lpi.astype(np.float32).astype(BF),
        l128_re=lam128.real.astype(np.float32),
        l128_im=lam128.imag.astype(np.float32),
        TT=np.triu(np.ones((C, C), np.float32)).astype(BF),
        WoT_re=np.ascontiguousarray(np.asarray(W_out_re, np.float32).T).astype(BF),
        WoT_imn=np.ascontiguousarray(-np.asarray(W_out_im, np.float32).T).astype(BF),
    )


def _bias_residual(params_log, b_in_re, b_in_im, W_out_re, W_out_im, b_out_re):
    """yb[t, d] = Re( (b~ * sum_{p<=t} lam^p) @ W_out^T ) + b_out  — the
    input-independent trajectory of b_in through scan + outproj."""
    nu, theta, gamma = np.exp(np.asarray(params_log, np.float64))
    loglam = -nu + 1j * theta                              # (H,)
    bt = (np.asarray(b_in_re, np.float64)
          + 1j * np.asarray(b_in_im, np.float64)) * gamma  # (H,)
    t = np.arange(1, L + 1)[:, None]                       # (L, 1): power t+1
    lam_tp1 = np.exp(t * loglam[None, :])                  # (L, H)
    geo = (1.0 - lam_tp1) / (1.0 - np.exp(loglam))[None, :]
    sb = bt[None, :] * geo                                 # (L, H) complex
    yb = (sb.real @ np.asarray(W_out_re, np.float64).T
          - sb.imag @ np.asarray(W_out_im, np.float64).T
          + np.asarray(b_out_re, np.float64)[None, :])
    return yb.astype(np.float32)                           # (L, D)


def _build(consts):
    import concourse.bass as bass
    import concourse.tile as tile
    from concourse import mybir

    f32 = mybir.dt.float32
    bf16 = mybir.dt.bfloat16
    AF = mybir.ActivationFunctionType
    OP = mybir.AluOpType

    nc = bass.Bass()
    x_in = nc.declare_dram_parameter("x", [L, D], f32, isOutput=False)
    xT_in = nc.declare_dram_parameter("xT", [128, NCHUNK, 4, 128], bf16, isOutput=False)
    y_out = nc.declare_dram_parameter("y", [L, D], f32, isOutput=True)

    dr = {k: nc.inline_tensor(v, name=f"c_{k}") for k, v in consts.items()}

    with tile.TileContext(nc) as tc:
        with tc.tile_pool(name="cst", bufs=1) as cst, \
             tc.tile_pool(name="ioT", bufs=3) as ioT, \
             tc.tile_pool(name="iox", bufs=6) as iox, \
             tc.tile_pool(name="ioo", bufs=3) as ioo, \
             tc.tile_pool(name="wk", bufs=3) as wk, \
             tc.tile_pool(name="sc", bufs=3) as sc, \
             tc.tile_pool(name="sm", bufs=4) as sm, \
             tc.tile_pool(name="psu", bufs=4, space="PSUM") as psu, \
             tc.tile_pool(name="psw", bufs=3, space="PSUM") as psw, \
             tc.tile_pool(name="psy", bufs=1, space="PSUM") as psy:

            # ---- load constants to SBUF ----
            # Issue order matters: only what inproj/premod of chunk 0 needs
            # goes first, so PE can start while the rest streams in.
            WtT = {}
            cst_t = {}

            def load_early_consts():
                for part in ("re", "im"):
                    for db in range(4):
                        t = cst.tile([128, H], bf16, name=f"WtT_{part}_{db}")
                        nc.sync.dma_start(out=t, in_=dr[f"WtT_{part}"][db * 128:(db + 1) * 128, :])
                        WtT[part, db] = t
                for k in ("lnr", "lni", "TT"):
                    shp = [128, H] if k != "TT" else [128, C]
                    cst_t[k] = cst.tile(shp, bf16, name=f"t_{k}")
                    nc.sync.dma_start(out=cst_t[k], in_=dr[k][:, :])

            eps_t = cst.tile([128, 1], f32, name="eps_t")
            nc.vector.memset(eps_t, EPS)

            def load_late_consts():
                WoT = {}
                for part, key in (("re", "WoT_re"), ("im", "WoT_imn")):
                    t = cst.tile([128, NBLK, D], bf16, name=f"WoT_{part}")
                    nc.sync.dma_start(out=t, in_=dr[key].rearrange("(b p) d -> p b d", p=128))
                    WoT[part] = t
                for k in ("lpr", "lpi"):
                    cst_t[k] = cst.tile([128, H], bf16, name=f"t_{k}")
                    nc.sync.dma_start(out=cst_t[k], in_=dr[k][:, :])
                for k in ("l128_re", "l128_im"):
                    cst_t[k] = cst.tile([128, NBLK], f32, name=f"t_{k}")
                    nc.sync.dma_start(out=cst_t[k], in_=dr[k][:, :])
                return WoT

            # persistent carry tiles, double-buffered by chunk parity
            c_t = {}
            for par in (0, 1):
                for part in ("re", "im"):
                    c_t[par, part] = cst.tile([128, NBLK], f32, name=f"carry_{par}_{part}")
            nc.vector.memset(c_t[0, "re"], 0.0)
            nc.vector.memset(c_t[0, "im"], 0.0)

            # per-chunk live tiles, keyed by chunk index
            xT_t, x_t, u_ps, u_sb, v_sb, w_ps, wp_sb, s_sb, y_ps = \
                {}, {}, {}, {}, {}, {}, {}, {}, {}

            def issue_load(k):
                r0 = k * C
                t = ioT.tile([128, 4, 128], bf16, tag="xT")
                nc.sync.dma_start(out=t, in_=xT_in[:, k, :, :])
                xT_t[k] = t
                t = iox.tile([128, D], f32, tag="x")
                nc.sync.dma_start(out=t, in_=x_in[r0:r0 + C, :])
                x_t[k] = t

            def issue_inproj(k):
                # db-outer so each LDWEIGHTS (stationary xT block) feeds 2 MMs
                for part in ("re", "im"):
                    pts = []
                    for nh in range(2):
                        pts.append(psu.tile([128, 512], f32, tag="psu", name="u_ps"))
                    for db in range(4):
                        for nh in range(2):
                            nc.tensor.matmul(pts[nh], xT_t[k][:, db, :],
                                             WtT[part, db][:, nh * 512:(nh + 1) * 512],
                                             start=(db == 0), stop=(db == 3))
                    u_ps[k, part] = pts

            def issue_uevict(k):
                for part in ("re", "im"):
                    t = wk.tile([128, H], bf16, tag=f"u_{part}")
                    for nh in range(2):
                        nc.scalar.activation(out=t[:, nh * 512:(nh + 1) * 512],
                                             in_=u_ps[k, part][nh], func=AF.Identity)
                    u_sb[k, part] = t

            def issue_premod(k):
                u = u_sb
                t1 = sc.tile([128, H], bf16, tag="t1")
                t2 = sc.tile([128, H], bf16, tag="t2")
                nc.vector.tensor_mul(t1, cst_t["lnr"], u[k, "re"])
                nc.vector.tensor_mul(t2, cst_t["lni"], u[k, "im"])
                v_re = wk.tile([128, H], bf16, tag="v_re")
                nc.vector.tensor_sub(v_re, t1, t2)
                t3 = sc.tile([128, H], bf16, tag="t3")
                t4 = sc.tile([128, H], bf16, tag="t4")
                nc.vector.tensor_mul(t3, cst_t["lnr"], u[k, "im"])
                nc.vector.tensor_mul(t4, cst_t["lni"], u[k, "re"])
                v_im = wk.tile([128, H], bf16, tag="v_im")
                nc.vector.tensor_add(v_im, t3, t4)
                v_sb[k] = {"re": v_re, "im": v_im}

            def issue_cumsum(k):
                # transposing cumsum: w^T[h, j] = sum_{q<=j} v[q, h]
                for part in ("re", "im"):
                    for half in range(2):
                        pt = psw.tile([128, 512], f32, tag="psw")
                        for bq in range(4):
                            b = half * 4 + bq
                            nc.tensor.matmul(pt[:, bq * 128:(bq + 1) * 128],
                                             v_sb[k][part][:, b * 128:(b + 1) * 128],
                                             cst_t["TT"], start=True, stop=True)
                        w_ps[k, part, half] = pt

            def issue_carry(k):
                # c_next = lam^128 * (w'[:,127] + c_cur); off the evict path
                cur, nxt = k % 2, (k + 1) % 2
                if k >= NCHUNK - 1:
                    return
                wcol = {}
                for part in ("re", "im"):
                    t = sm.tile([128, NBLK], f32, tag=f"wcol_{part}")
                    for half in range(2):
                        nc.vector.tensor_add(
                            t[:, half * 4:(half + 1) * 4],
                            w_ps[k, part, half][:, 127::128],
                            c_t[cur, part][:, half * 4:(half + 1) * 4])
                    wcol[part] = t
                m1 = sm.tile([128, NBLK], f32, tag="m1")
                m2 = sm.tile([128, NBLK], f32, tag="m2")
                nc.vector.tensor_mul(m1, cst_t["l128_re"], wcol["re"])
                nc.vector.tensor_mul(m2, cst_t["l128_im"], wcol["im"])
                nc.vector.tensor_sub(c_t[nxt, "re"], m1, m2)
                m3 = sm.tile([128, NBLK], f32, tag="m3")
                m4 = sm.tile([128, NBLK], f32, tag="m4")
                nc.vector.tensor_mul(m3, cst_t["l128_re"], wcol["im"])
                nc.vector.tensor_mul(m4, cst_t["l128_im"], wcol["re"])
                nc.vector.tensor_add(c_t[nxt, "im"], m3, m4)

            def issue_wevict(k):
                # carry-add + evict (ACT Identity w/ per-partition bias)
                cur = k % 2
                for part in ("re", "im"):
                    t = wk.tile([128, H], bf16, tag=f"wp_{part}")
                    for b in range(NBLK):
                        nc.scalar.activation(
                            out=t[:, b * 128:(b + 1) * 128],
                            in_=w_ps[k, part, b // 4][:, (b % 4) * 128:(b % 4 + 1) * 128],
                            func=AF.Identity,
                            bias=c_t[cur, part][:, b:b + 1])
                    wp_sb[k, part] = t

            def issue_postmod(k):
                p1 = sc.tile([128, H], bf16, tag="p1")
                p2 = sc.tile([128, H], bf16, tag="p2")
                nc.vector.tensor_mul(p1, cst_t["lpr"], wp_sb[k, "re"])
                nc.vector.tensor_mul(p2, cst_t["lpi"], wp_sb[k, "im"])
                s_re = wk.tile([128, H], bf16, tag="s_re")
                nc.vector.tensor_sub(s_re, p1, p2)
                p3 = sc.tile([128, H], bf16, tag="p3")
                p4 = sc.tile([128, H], bf16, tag="p4")
                nc.vector.tensor_mul(p3, cst_t["lpr"], wp_sb[k, "im"])
                nc.vector.tensor_mul(p4, cst_t["lpi"], wp_sb[k, "re"])
                s_im = wk.tile([128, H], bf16, tag="s_im")
                nc.vector.tensor_add(s_im, p3, p4)
                s_sb[k] = {"re": s_re, "im": s_im}

            def issue_outproj(k):
                pt = psy.tile([128, 512], f32, tag="psy")
                first = True
                for part in ("re", "im"):
                    for b in range(NBLK):
                        nc.tensor.matmul(pt, s_sb[k][part][:, b * 128:(b + 1) * 128],
                                         WoT[part][:, b, :], start=first,
                                         stop=(part == "im" and b == NBLK - 1))
                        first = False
                y_ps[k] = pt

            def issue_ln(k):
                r0 = k * C
                ysum = sm.tile([128, 1], f32, tag="ysum")
                y_sb = iox.tile([128, D], f32, tag="ysb")
                nc.vector.scalar_tensor_tensor(out=y_sb, in0=y_ps[k], scalar=0.0,
                                               in1=x_t[k], op0=OP.bypass, op1=OP.add,
                                               accum_out=ysum)
                negmu = sm.tile([128, 1], f32, tag="negmu")
                nc.scalar.mul(negmu, ysum, -1.0 / D)
                sq = sc.tile([128, D], bf16, tag="sq")
                ss = sm.tile([128, 1], f32, tag="ss")
                nc.scalar.activation(out=sq, in_=y_sb, func=AF.Square,
                                     bias=negmu, accum_out=ss)
                sd = sm.tile([128, 1], f32, tag="sd")
                nc.scalar.activation(out=sd, in_=ss, func=AF.Sqrt,
                                     bias=eps_t, scale=1.0 / D)
                rstd = sm.tile([128, 1], f32, tag="rstd")
                nc.vector.reciprocal(rstd, sd)
                nmr = sm.tile([128, 1], f32, tag="nmr")
                nc.vector.tensor_mul(nmr, negmu, rstd)
                o_sb = ioo.tile([128, D], f32, tag="osb")
                nc.scalar.activation(out=o_sb, in_=y_sb, func=AF.Identity,
                                     bias=nmr, scale=rstd)
                nc.sync.dma_start(out=y_out[r0:r0 + C, :], in_=o_sb)

            issue_load(0)
            load_early_consts()
            issue_load(1)
            WoT = load_late_consts()
            for it in range(NCHUNK + 2):
                if it < NCHUNK:
                    if it + 2 < NCHUNK:
                        issue_load(it + 2)
                    issue_inproj(it)
                    issue_uevict(it)
                    issue_premod(it)
                k1 = it - 1
                if 0 <= k1 < NCHUNK:
                    issue_cumsum(k1)
                    issue_carry(k1)
                    issue_wevict(k1)
                    issue_postmod(k1)
                k2 = it - 2
                if 0 <= k2 < NCHUNK:
                    issue_outproj(k2)
                    issue_ln(k2)

    orig = type(nc).to_json_bytes
    nc.to_json_bytes = lambda: _split_multi_waits(orig(nc))
    return nc


def _reference_fallback(x, mask, params_log, W_in_re, W_in_im, b_in_re, b_in_im,
                        W_out_re, W_out_im, b_out_re, b_out_im, ln_w, ln_b):
    nu, theta, gamma = np.exp(np.asarray(params_log, np.float64))
    lam = np.exp(-nu + 1j * theta).astype(np.complex64)[None, :]
    W_in = (np.asarray(W_in_re) + 1j * np.asarray(W_in_im)).astype(np.complex64)
    b_in = (np.asarray(b_in_re) + 1j * np.asarray(b_in_im)).astype(np.complex64)
    W_out = (np.asarray(W_out_re) + 1j * np.asarray(W_out_im)).astype(np.complex64)
    h = (np.asarray(x).astype(np.complex64) @ W_in.T + b_in) * gamma.astype(np.float32)
    Bn, Ln, Dn = h.shape
    log2_L = int(np.ceil(np.log2(Ln)))
    m_ = np.asarray(mask, np.float32)
    lamb = lam.copy()
    for i in range(1, log2_L + 1):
        l = 2 ** i
        hh = h.reshape(Bn * Ln // l, l, Dn)
        mm = m_.reshape(Bn * Ln // l, l)
        h1, h2 = hh[:, :l // 2], hh[:, l // 2:]
        if i > 1:
            lamb = np.concatenate([lamb, lamb * lamb[-1]], axis=0)
        h2 = h2 + lamb * h1[:, -1:] * mm[:, l // 2 - 1:l // 2, None]
        h = np.concatenate([h1, h2], axis=1)
    h = h.reshape(Bn, Ln, Dn)
    y = (h @ W_out.T + (np.asarray(b_out_re) + 1j * np.asarray(b_out_im))).real + np.asarray(x)
    mean = y.mean(-1, keepdims=True)
    var = y.var(-1, keepdims=True)
    return ((y - mean) / np.sqrt(var + EPS) * np.asarray(ln_w) + np.asarray(ln_b)).astype(np.float32)


def kernel(x, mask, params_log, W_in_re, W_in_im, b_in_re, b_in_im,
           W_out_re, W_out_im, b_out_re, b_out_im, ln_w, ln_b, **_):
    x = np.asarray(x, np.float32)
    if not (np.all(np.asarray(mask) == 1.0) and np.allclose(ln_w, 1.0)
            and np.allclose(ln_b, 0.0)):
        return _reference_fallback(x, mask, params_log, W_in_re, W_in_im, b_in_re,
                                   b_in_im, W_out_re, W_out_im, b_out_re, b_out_im,
                                   ln_w, ln_b)
    from concourse.bass_utils import run_bass_kernel_spmd

    consts = _prep_consts(params_log, W_in_re, W_in_im, W_out_re, W_out_im)
    yb = _bias_residual(params_log, b_in_re, b_in_im, W_out_re, W_out_im, b_out_re)

    key = hash((np.asarray(params_log, np.float64).tobytes(),
                np.asarray(W_in_re, np.float32).tobytes(),
                np.asarray(W_out_re, np.float32).tobytes()))
    nc = _COMPILED.get(key)
    if nc is None:
        nc = _build(consts)
        _COMPILED[key] = nc

    in_maps = []
    for b in range(B):
        xT = np.ascontiguousarray(
            x[b].T.reshape(4, 128, NCHUNK, 128).transpose(1, 2, 0, 3)).astype(BF)
        in_maps.append({"x": np.ascontiguousarray(x[b] + yb), "xT": xT})
    res = run_bass_kernel_spmd(nc, in_maps, core_ids=list(range(B)))
    return np.stack([res.results[b]["y"] for b in range(B)]).astype(np.float32)
